# revision 27
# baseline (speedup 1.0000x reference)
"""ArcFace logits kernel for 8 TRN2 NeuronCores (partial-FC tensor parallel).

logits = scale * where(one_hot(labels), cos(arccos(cosine)+m), cosine)
  cosine = normalize(emb) @ normalize(W)   [B=512, C=100000]

Sharding: W columns (and the [B, C] output) split across 8 cores, 12500
columns each; embeddings broadcast. No collectives.

Split of work:
  host   - L2-normalize W columns / emb rows in f32, scale emb by 64,
           cast both to bf16, pack into per-core DMA-friendly layouts;
           after the device pass, patch the B label entries with the
           f32-exact margin value (cos(arccos(c)+m)*64) and cast the
           bf16 result tile back to f32.
  device - pure streaming GEMM: per 500-column sub-block, one 0.5 MB
           HWDGE load (sync ring), 16 bf16 matmuls (4 batch tiles x 4
           K tiles) accumulating in PSUM, 4 DVE psum->sbuf bf16 evict
           copies, one 0.5 MB HWDGE store (scalar ring).

Per-core traffic ~26 MB (bf16 both ways) and ~85 us of PE time at
N=500 stream rate; the kernel runs at the PE roofline with DMA fully
overlapped (zero matmul stalls in steady state).

TRN2 specifics that shaped the head/tail:
  - HAM clock-gate: the PE runs at 1.2 GHz until one 4096-cycle window
    is FULLY busy, then 2.4 GHz. A stream of throwaway warm-up matmuls
    covers the pipeline-fill window so real matmuls start at full rate;
    a drip-fed start (gappy stream) must be avoided.
  - The first matmul gates on wb0 (solo first on the SP ring) plus the
    first 128 KB bt-piece of e64, not the full megabyte.
  - The last store is split per batch tile so the final HBM write
    receipt isn't behind one serialized 0.5 MB transfer.
"""

import math

import numpy as np

import concourse.bass as bass
import concourse.tile as tile
from concourse import mybir
from concourse.bass_utils import run_bass_kernel_spmd

N_CORES = 8
B = 512          # batch
D = 512          # embed dim
C = 100000       # num classes
CS = C // N_CORES          # 12500 columns per core
SUB = 500                  # matmul/DMA sub-block width
NSUB = CS // SUB           # 25
DT = D // 128
BT = B // 128
SCALE = 64.0
MARGIN = 0.5
EPS = 1e-7
F32 = mybir.dt.float32
BF16 = mybir.dt.bfloat16

_MAX_WAITS = 1


def _legalize_waits(nc, max_waits=_MAX_WAITS):
    """Split multi-wait instructions for this toolchain's codegen.

    The pinned neuronxcc rejects instructions carrying more than one sync
    wait ("Too many sync wait commands" in setupSyncWait). Tile's semaphore
    assignment can attach several waits to one instruction. Hoist the
    overflow onto no-op instructions emitted just before, on the same
    engine — the engine blocks on those first, which is semantically
    identical.
    """
    n = 0
    for fn in nc.m.functions:
        for bb in fn.blocks:
            out = []
            for inst in bb.instructions:
                si = inst.sync_info
                if si is not None and si.on_wait and len(si.on_wait) > max_waits:
                    waits = list(si.on_wait)
                    keep = waits[-max_waits:]
                    over = waits[:-max_waits]
                    for i in range(0, len(over), max_waits):
                        nop = mybir.InstNoOp(
                            name=f"waitsplit_{n}",
                            sync_info=mybir.SyncInfo(
                                on_wait=over[i : i + max_waits], on_update=[]
                            ),
                            bass_nofuse=True,
                            engine=inst.engine,
                        )
                        n += 1
                        nc.register_instruction(nop)
                        out.append(nop)
                    inst.sync_info = mybir.SyncInfo(
                        on_wait=keep, on_update=list(si.on_update or [])
                    )
                out.append(inst)
            bb.instructions[:] = out
    return n


def build(prefetch=6, w_bufs=8, out_bufs=4, ps_bufs=8, n_warm=18):
    nc = bass.Bass("TRN2", target_bir_lowering=False, debug=False, num_devices=N_CORES)
    w_ext = nc.declare_dram_parameter("w", [NSUB, 128, DT * SUB], BF16, isOutput=False)
    e_ext = nc.declare_dram_parameter("e64", [128, DT * B], BF16, isOutput=False)
    out_ext = nc.declare_dram_parameter("out", [NSUB, 128, BT * SUB], BF16, isOutput=True)

    w_ap = w_ext.ap()      # [NSUB, 128, DT*SUB]
    # bt-major: [128, BT, DT, 128] so per-bt pieces are contiguous
    e_ap = e_ext.ap().rearrange("p (t a c) -> p t a c", t=BT, a=DT)
    out_ap = out_ext.ap()  # [NSUB, 128, BT*SUB]

    with tile.TileContext(nc) as tc:
        with (
            tc.tile_pool(name="persist", bufs=1) as persist,
            tc.tile_pool(name="ps", bufs=ps_bufs, space="PSUM") as psp,
            tc.tile_pool(name="wp", bufs=w_bufs) as wp,
            tc.tile_pool(name="op", bufs=out_bufs) as op,
        ):
            # HAM warm-up: run throwaway matmuls over a scratch tile while
            # the first loads are in flight, so the PE clock-gate is at
            # 8/8 (2.4 GHz) by the time real matmuls start. The memset
            # rides the otherwise-idle GpSimd queue.
            warm = persist.tile([128, 384], BF16)
            nc.gpsimd.memset(warm[:], 0)

            # wb0 alone first on the SP ring (drains at full SDMA rate,
            # no round-robin with a concurrent queue), then e64 behind it
            # in per-bt pieces: bt0's matmuls gate on wb0 + 128 KB, not
            # the full e64, and later pieces land ahead of consumption.
            e64 = persist.tile([128, BT, DT, 128], BF16)

            wb_t = {}

            def load(s):
                wb = wp.tile([128, DT, SUB], BF16, tag="wb")
                nc.sync.dma_start(out=wb[:], in_=w_ap[s])
                wb_t[s] = wb

            # wb0 whole-tile first on the SP ring, then e64 per-bt pieces
            # split across both HWDGE rings. Piecewise (drip-fed) first
            # blocks were measured SLOWER: arrivals can't keep up with the
            # 213ns/MM consumption, the resulting stream gaps defeat the
            # HAM fully-busy-window flip, and a stretch of matmuls runs at
            # 1.2 GHz — late-but-contiguous beats early-but-gappy.
            load(0)
            for bt in range(BT):
                eng = nc.sync if bt < 2 else nc.scalar
                eng.dma_start(out=e64[:, bt], in_=e_ap[:, bt])
            pw = psp.tile([128, 256], F32, tag="pm")
            for _ in range(n_warm):
                nc.tensor.matmul(
                    pw[:], lhsT=warm[:, :128], rhs=warm[:, 128:384],
                    start=True, stop=True,
                )
            for s in range(1, min(prefetch, NSUB)):
                load(s)
            for s in range(NSUB):
                if s + prefetch < NSUB:
                    load(s + prefetch)
                wb = wb_t.pop(s)
                last = s == NSUB - 1
                outc = op.tile([128, BT, SUB], BF16, tag="outc")
                for bt in range(BT):
                    pm = psp.tile([128, SUB], F32, tag="pm")
                    for d in range(DT):
                        nc.tensor.matmul(
                            pm[:],
                            lhsT=e64[:, bt, d, :],
                            rhs=wb[:, d, :],
                            start=(d == 0),
                            stop=(d == DT - 1),
                        )
                    nc.vector.tensor_copy(outc[:, bt, :], pm[:])
                    if last:
                        # drain the tail at bt granularity, dispatches
                        # alternating across both HWDGE rings (SP is idle
                        # by now), so the final store + its HBM write
                        # receipt start as early as possible. Finer splits
                        # measured slower: per-dispatch cost (~0.6us)
                        # outweighs the smaller final piece.
                        eng = nc.scalar if bt % 2 == 0 else nc.sync
                        eng.dma_start(
                            out=out_ap[s][:, bt * SUB : (bt + 1) * SUB],
                            in_=outc[:, bt, :],
                        )
                if not last:
                    nc.scalar.dma_start(out=out_ap[s], in_=outc[:])

    _legalize_waits(nc)
    return nc


def _host_prep(embeddings, labels, class_weights):
    embeddings = np.asarray(embeddings, dtype=np.float32)
    labels = np.asarray(labels).astype(np.int64)
    class_weights = np.asarray(class_weights, dtype=np.float32)
    bf16 = mybir.dt.np(BF16)

    # normalized embeddings (f32) and the 64x-scaled bf16 operand,
    # packed bt-major: e_packed[p, bt, d, c] = 64*emb_n[bt*128+c, d*128+p]
    emb_n = embeddings / np.linalg.norm(embeddings, axis=1, keepdims=True)
    e64 = (SCALE * emb_n).T                                   # [D, B]
    e_packed = np.ascontiguousarray(
        e64.reshape(DT, 128, BT, 128)
        .transpose(1, 2, 0, 3)
        .reshape(128, DT * B)
        .astype(bf16)
    )

    # normalized class weights (f32) -> bf16, packed per core as
    # [NSUB, 128, DT*SUB] with element [s, p, d*SUB+c] = Wn[d*128+p, ...]
    w_n = class_weights / np.linalg.norm(class_weights, axis=0, keepdims=True)
    w_pack = (
        w_n.reshape(DT, 128, N_CORES, NSUB, SUB)
        .transpose(2, 3, 1, 0, 4)
        .reshape(N_CORES, NSUB, 128, DT * SUB)
        .astype(bf16)
    )

    # f32-exact margin fix values for the label entries
    cos_lab = np.einsum("bd,db->b", emb_n, w_n[:, labels]).astype(np.float32)
    cos_lab = np.clip(cos_lab, -1.0 + EPS, 1.0 - EPS)
    target = (SCALE * np.cos(np.arccos(cos_lab) + MARGIN)).astype(np.float32)

    in_maps = [
        {"w": np.ascontiguousarray(w_pack[core]), "e64": e_packed}
        for core in range(N_CORES)
    ]
    return labels, target, in_maps


def kernel(embeddings, labels, class_weights, _trace=False):
    labels, target, in_maps = _host_prep(embeddings, labels, class_weights)
    nc = build()
    res = run_bass_kernel_spmd(
        nc, in_maps, core_ids=list(range(N_CORES)), trace=_trace
    )
    # gather: out[core] is [NSUB, 128, BT*SUB]; row b = bt*128+p,
    # col = core*CS + s*SUB + c
    packed = np.stack([res.results[i]["out"] for i in range(N_CORES)])
    full = (
        packed.reshape(N_CORES, NSUB, 128, BT, SUB)
        .transpose(3, 2, 0, 1, 4)
        .reshape(B, C)
        .astype(np.float32)
    )
    full[np.arange(B), labels] = target
    if _trace:
        kernel.last_results = res
    return full


# revision 28
# speedup vs baseline: 1.0258x; 1.0258x over previous
"""ArcFace logits kernel for 8 TRN2 NeuronCores (partial-FC tensor parallel).

logits = scale * where(one_hot(labels), cos(arccos(cosine)+m), cosine)
  cosine = normalize(emb) @ normalize(W)   [B=512, C=100000]

Sharding: W columns (and the [B, C] output) split across 8 cores, 12500
columns each; embeddings broadcast. No collectives.

Split of work:
  host   - L2-normalize W columns / emb rows in f32, scale emb by 64,
           cast both to bf16, pack into per-core DMA-friendly layouts;
           after the device pass, patch the B label entries with the
           f32-exact margin value (cos(arccos(c)+m)*64) and cast the
           bf16 result tile back to f32.
  device - pure streaming GEMM: per 500-column sub-block, one 0.5 MB
           HWDGE load (sync ring), 16 bf16 matmuls (4 batch tiles x 4
           K tiles) accumulating in PSUM, 4 DVE psum->sbuf bf16 evict
           copies, one 0.5 MB HWDGE store (scalar ring).

Per-core traffic ~26 MB (bf16 both ways) and ~85 us of PE time at
N=500 stream rate; the kernel runs at the PE roofline with DMA fully
overlapped (zero matmul stalls in steady state).

TRN2 specifics that shaped the head/tail:
  - HAM clock-gate: the PE runs at 1.2 GHz until one 4096-cycle window
    is FULLY busy, then 2.4 GHz. A stream of throwaway warm-up matmuls
    covers the pipeline-fill window so real matmuls start at full rate;
    a drip-fed start (gappy stream) must be avoided.
  - The first matmul gates on wb0 (solo first on the SP ring) plus the
    first 128 KB bt-piece of e64, not the full megabyte.
  - The last store is split per batch tile so the final HBM write
    receipt isn't behind one serialized 0.5 MB transfer.
"""

import math

import numpy as np

import concourse.bass as bass
import concourse.tile as tile
from concourse import mybir
from concourse.bass_utils import run_bass_kernel_spmd

N_CORES = 8
B = 512          # batch
D = 512          # embed dim
C = 100000       # num classes
CS = C // N_CORES          # 12500 columns per core
SUB = 500                  # matmul/DMA sub-block width
NSUB = CS // SUB           # 25
DT = D // 128
BT = B // 128
SCALE = 64.0
MARGIN = 0.5
EPS = 1e-7
F32 = mybir.dt.float32
BF16 = mybir.dt.bfloat16

_MAX_WAITS = 1


def _legalize_waits(nc, max_waits=_MAX_WAITS):
    """Split multi-wait instructions for this toolchain's codegen.

    The pinned neuronxcc rejects instructions carrying more than one sync
    wait ("Too many sync wait commands" in setupSyncWait). Tile's semaphore
    assignment can attach several waits to one instruction. Hoist the
    overflow onto no-op instructions emitted just before, on the same
    engine — the engine blocks on those first, which is semantically
    identical.
    """
    n = 0
    for fn in nc.m.functions:
        for bb in fn.blocks:
            out = []
            for inst in bb.instructions:
                si = inst.sync_info
                if si is not None and si.on_wait and len(si.on_wait) > max_waits:
                    waits = list(si.on_wait)
                    keep = waits[-max_waits:]
                    over = waits[:-max_waits]
                    for i in range(0, len(over), max_waits):
                        nop = mybir.InstNoOp(
                            name=f"waitsplit_{n}",
                            sync_info=mybir.SyncInfo(
                                on_wait=over[i : i + max_waits], on_update=[]
                            ),
                            bass_nofuse=True,
                            engine=inst.engine,
                        )
                        n += 1
                        nc.register_instruction(nop)
                        out.append(nop)
                    inst.sync_info = mybir.SyncInfo(
                        on_wait=keep, on_update=list(si.on_update or [])
                    )
                out.append(inst)
            bb.instructions[:] = out
    return n


def build(prefetch=9, w_bufs=11, out_bufs=6, ps_bufs=8, n_warm=18):
    nc = bass.Bass("TRN2", target_bir_lowering=False, debug=False, num_devices=N_CORES)
    w_ext = nc.declare_dram_parameter("w", [NSUB, 128, DT * SUB], BF16, isOutput=False)
    e_ext = nc.declare_dram_parameter("e64", [128, DT * B], BF16, isOutput=False)
    out_ext = nc.declare_dram_parameter("out", [NSUB, 128, BT * SUB], BF16, isOutput=True)

    w_ap = w_ext.ap()      # [NSUB, 128, DT*SUB]
    # bt-major: [128, BT, DT, 128] so per-bt pieces are contiguous
    e_ap = e_ext.ap().rearrange("p (t a c) -> p t a c", t=BT, a=DT)
    out_ap = out_ext.ap()  # [NSUB, 128, BT*SUB]

    with tile.TileContext(nc) as tc:
        with (
            tc.tile_pool(name="persist", bufs=1) as persist,
            tc.tile_pool(name="ps", bufs=ps_bufs, space="PSUM") as psp,
            tc.tile_pool(name="wp", bufs=w_bufs) as wp,
            tc.tile_pool(name="op", bufs=out_bufs) as op,
        ):
            # HAM warm-up: run throwaway matmuls over a scratch tile while
            # the first loads are in flight, so the PE clock-gate is at
            # 8/8 (2.4 GHz) by the time real matmuls start. The memset
            # rides the otherwise-idle GpSimd queue.
            warm = persist.tile([128, 384], BF16)
            nc.gpsimd.memset(warm[:], 0)

            # wb0 alone first on the SP ring (drains at full SDMA rate,
            # no round-robin with a concurrent queue), then e64 behind it
            # in per-bt pieces: bt0's matmuls gate on wb0 + 128 KB, not
            # the full e64, and later pieces land ahead of consumption.
            e64 = persist.tile([128, BT, DT, 128], BF16)

            wb_t = {}

            def load(s):
                wb = wp.tile([128, DT, SUB], BF16, tag="wb")
                nc.sync.dma_start(out=wb[:], in_=w_ap[s])
                wb_t[s] = wb

            # wb0 whole-tile first on the SP ring, then e64 per-bt pieces
            # split across both HWDGE rings. Piecewise (drip-fed) first
            # blocks were measured SLOWER: arrivals can't keep up with the
            # 213ns/MM consumption, the resulting stream gaps defeat the
            # HAM fully-busy-window flip, and a stretch of matmuls runs at
            # 1.2 GHz — late-but-contiguous beats early-but-gappy.
            load(0)
            for bt in range(BT):
                eng = nc.sync if bt < 2 else nc.scalar
                eng.dma_start(out=e64[:, bt], in_=e_ap[:, bt])
            pw = psp.tile([128, 256], F32, tag="pm")
            for _ in range(n_warm):
                nc.tensor.matmul(
                    pw[:], lhsT=warm[:, :128], rhs=warm[:, 128:384],
                    start=True, stop=True,
                )
            for s in range(1, min(prefetch, NSUB)):
                load(s)
            for s in range(NSUB):
                if s + prefetch < NSUB:
                    load(s + prefetch)
                wb = wb_t.pop(s)
                last = s == NSUB - 1
                outc = op.tile([128, BT, SUB], BF16, tag="outc")
                for bt in range(BT):
                    pm = psp.tile([128, SUB], F32, tag="pm")
                    for d in range(DT):
                        nc.tensor.matmul(
                            pm[:],
                            lhsT=e64[:, bt, d, :],
                            rhs=wb[:, d, :],
                            start=(d == 0),
                            stop=(d == DT - 1),
                        )
                    nc.vector.tensor_copy(outc[:, bt, :], pm[:])
                    if last:
                        # drain the tail at bt granularity, dispatches
                        # alternating across both HWDGE rings (SP is idle
                        # by now), so the final store + its HBM write
                        # receipt start as early as possible. Finer splits
                        # measured slower: per-dispatch cost (~0.6us)
                        # outweighs the smaller final piece.
                        eng = nc.scalar if bt % 2 == 0 else nc.sync
                        eng.dma_start(
                            out=out_ap[s][:, bt * SUB : (bt + 1) * SUB],
                            in_=outc[:, bt, :],
                        )
                if not last:
                    nc.scalar.dma_start(out=out_ap[s], in_=outc[:])

    _legalize_waits(nc)
    return nc


def _host_prep(embeddings, labels, class_weights):
    embeddings = np.asarray(embeddings, dtype=np.float32)
    labels = np.asarray(labels).astype(np.int64)
    class_weights = np.asarray(class_weights, dtype=np.float32)
    bf16 = mybir.dt.np(BF16)

    # normalized embeddings (f32) and the 64x-scaled bf16 operand,
    # packed bt-major: e_packed[p, bt, d, c] = 64*emb_n[bt*128+c, d*128+p]
    emb_n = embeddings / np.linalg.norm(embeddings, axis=1, keepdims=True)
    e64 = (SCALE * emb_n).T                                   # [D, B]
    e_packed = np.ascontiguousarray(
        e64.reshape(DT, 128, BT, 128)
        .transpose(1, 2, 0, 3)
        .reshape(128, DT * B)
        .astype(bf16)
    )

    # normalized class weights (f32) -> bf16, packed per core as
    # [NSUB, 128, DT*SUB] with element [s, p, d*SUB+c] = Wn[d*128+p, ...]
    w_n = class_weights / np.linalg.norm(class_weights, axis=0, keepdims=True)
    w_pack = (
        w_n.reshape(DT, 128, N_CORES, NSUB, SUB)
        .transpose(2, 3, 1, 0, 4)
        .reshape(N_CORES, NSUB, 128, DT * SUB)
        .astype(bf16)
    )

    # f32-exact margin fix values for the label entries
    cos_lab = np.einsum("bd,db->b", emb_n, w_n[:, labels]).astype(np.float32)
    cos_lab = np.clip(cos_lab, -1.0 + EPS, 1.0 - EPS)
    target = (SCALE * np.cos(np.arccos(cos_lab) + MARGIN)).astype(np.float32)

    in_maps = [
        {"w": np.ascontiguousarray(w_pack[core]), "e64": e_packed}
        for core in range(N_CORES)
    ]
    return labels, target, in_maps


def kernel(embeddings, labels, class_weights, _trace=False):
    labels, target, in_maps = _host_prep(embeddings, labels, class_weights)
    nc = build()
    res = run_bass_kernel_spmd(
        nc, in_maps, core_ids=list(range(N_CORES)), trace=_trace
    )
    # gather: out[core] is [NSUB, 128, BT*SUB]; row b = bt*128+p,
    # col = core*CS + s*SUB + c
    packed = np.stack([res.results[i]["out"] for i in range(N_CORES)])
    full = (
        packed.reshape(N_CORES, NSUB, 128, BT, SUB)
        .transpose(3, 2, 0, 1, 4)
        .reshape(B, C)
        .astype(np.float32)
    )
    full[np.arange(B), labels] = target
    if _trace:
        kernel.last_results = res
    return full
